# revision 19
# baseline (speedup 1.0000x reference)
"""PointConvolution (8-neighbor shifted diffs + 1x1 conv) as a single 3x3 conv,
run data-parallel across 8 TRN2 NeuronCores via Bass/Tile.

Math: out[o,h,w] = sum_k sum_c W[o,3k+c] * (xpad[c,h+ik,w+jk] - x[c,h,w]) + b[o]
    = sum_{c,i,j} K3[o,c,i,j] * xpad[c,h+i,w+j] + b[o]
  where K3 gets W at the 8 non-center taps and -sum(W over taps) at center.

Device scheme per core (2 images), v4 (K=82, one matmul per row-group):
  - Output rows in chunks of 32 (TB=8 groups of G=4 rows).
  - M=128 PSUM partitions = (g in 0..3, o in 0..31); contraction partitions
    32j + (6c+s) for kernel column j in 0..2, folded in via shifted replicas.
  - Host pre-gathers each chunk's input window into [18, TB*Wp] (im2row over
    rows); one DMA loads it at partitions 0-17, then two ACT copies build the
    j=1,2 replicas at partitions 32/64, shifted 1/2 cols (ACT is the only
    engine that may write float32r, and partition starts must be 32-aligned).
    Gap partitions 18-31/50-63 are zeroed once per pool buffer at startup so
    the matmul (zero weight rows there) never sees NaN garbage.
  - Per group t: ONE self-loading fp32r matmul [82x128]x[82x512] (start+stop).
  - PSUM -> SBUF drain: one DVE tensor_scalar_add per 4-bank half (adds bias),
    then one 128x16KB-descriptor DMA per chunk writes a permuted DRAM layout;
    host transposes during unshard. DMA rings only carry the 9.5MB input +
    67MB output (+ ~214us/core HBM roofline at 358 GB/s).
"""

import numpy as np

import concourse.bacc as bacc
import concourse.bass as bass
import concourse.tile as tile
from concourse import mybir
from concourse.bass_utils import run_bass_kernel_spmd

# Problem constants (hardcoded per harness contract)
B, C, H, W_DIM, OUT = 16, 3, 512, 512, 32
KS, P = 3, 1
NCORES = 8
NB = B // NCORES          # images per core = 2
Hp, Wp = H + 2 * P, W_DIM + 2 * P   # 514, 514

G = 4                     # output rows per matmul group
S = G + KS - 1            # input rows per group window = 6
T = 4                     # groups per PSUM half
TB = 8                    # groups per chunk (32 output rows)
CH = G * TB               # 32 output rows per chunk
NCHUNK = H // CH          # 16 chunks per image
K0 = C * S                # 18 base contraction partitions
JSTEP = 32                # partition stride between j-blocks (alignment)
K = 2 * JSTEP + K0        # 82 contraction partitions incl. zero gaps
GAP = JSTEP - K0          # 14 zero partitions after each of j=0,1 blocks
M = G * OUT               # 128 output partitions
FW = TB * Wp              # 4112 free cols per contraction row
OBF = 2 * T * W_DIM       # 4096 free cols in the output tile
XBUFS = 4                 # xin pool depth (warmup-zeroed once per buffer)

F32 = mybir.dt.float32
MM_DT = mybir.dt.float32r  # PE streams 1 cyc/row (vs 4 for plain fp32)


def _round_fp32r(a):
    """fp32r = fp32 with mantissa rounded (RNE) to 11 bits, low 12 bits zero."""
    bits = np.ascontiguousarray(a, np.float32).view(np.uint32)
    lsb = (bits >> np.uint32(12)) & np.uint32(1)
    out = (bits + np.uint32(0x7FF) + lsb) & np.uint32(0xFFFFF000)
    return out.view(np.float32)


def _coords():
    i, j = np.meshgrid(np.arange(KS), np.arange(KS))
    coords = np.dstack((i.reshape(-1), j.reshape(-1)))[0]
    return coords[np.any(coords != P, axis=1)]


def _build_weights(W, b):
    K3 = np.zeros((OUT, C, KS, KS), np.float32)
    Wr = W.reshape(OUT, 8, C)
    for k, (i, j) in enumerate(_coords()):
        K3[:, :, i, j] += Wr[:, k, :]
    K3[:, :, P, P] = -Wr.sum(axis=1)

    # wt[32j + Sc + s, 32g + o] = K3[o, c, s-g, j] when 0 <= s-g < KS
    wt = np.zeros((K, M), np.float32)
    for j in range(KS):
        for c in range(C):
            for s in range(S):
                for g in range(G):
                    i = s - g
                    if 0 <= i < KS:
                        wt[JSTEP * j + S * c + s, OUT * g: OUT * (g + 1)] = K3[:, c, i, j]
    bias = np.tile(b.astype(np.float32), G).reshape(M, 1)
    return wt, bias


def _build_xin(x):
    """[B,C,H,W] -> [B, NCHUNK, K0, TB*Wp] im2row over rows (padding embedded)."""
    xpad = np.pad(np.ascontiguousarray(x, np.float32),
                  ((0, 0), (0, 0), (P, P), (P, P)))
    ch = np.arange(NCHUNK)[:, None, None]
    s = np.arange(S)[None, :, None]
    t = np.arange(TB)[None, None, :]
    rows = CH * ch + G * t + s                      # [NCHUNK, S, TB]
    big = xpad[:, :, rows, :]                       # [B, C, NCHUNK, S, TB, Wp]
    big = big.transpose(0, 2, 1, 3, 4, 5)           # [B, NCHUNK, C, S, TB, Wp]
    return np.ascontiguousarray(big).reshape(B, NCHUNK, K0, FW)


def _build_bass():
    # Bacc (not plain Bass): its compile() runs move_matmul_waits_to_ldweights
    # and generate_event_semaphores, required because TRN2 instructions take
    # at most one semaphore wait.
    nc = bacc.Bacc("TRN2")
    x_d = nc.declare_dram_parameter("xin", [NB, NCHUNK, K0, FW], MM_DT, isOutput=False)
    wt_d = nc.declare_dram_parameter("wt", [K, M], MM_DT, isOutput=False)
    b_d = nc.declare_dram_parameter("bias", [M, 1], F32, isOutput=False)
    z_d = nc.declare_dram_parameter("zeros", [GAP, FW], MM_DT, isOutput=False)
    out_d = nc.declare_dram_parameter("out", [NB, NCHUNK, M, OBF], F32, isOutput=True)

    with tile.TileContext(nc) as tc:
        with (
            tc.tile_pool(name="wpool", bufs=1) as wpool,
            tc.tile_pool(name="xpool", bufs=XBUFS) as xpool,
            tc.tile_pool(name="opool", bufs=4) as opool,
            tc.tile_pool(name="psum", bufs=2, space=bass.MemorySpace.PSUM) as ppool,
        ):
            wsb = wpool.tile([K, M], MM_DT)
            nc.scalar.dma_start(wsb[:], wt_d[:])
            bsb = wpool.tile([M, 1], F32)
            nc.scalar.dma_start(bsb[:], b_d[:])

            # Zero the gap partitions of each pool buffer once; the loop never
            # writes them, so they stay zero for every later chunk.
            zsrc = bass.AP(z_d, 0, [[FW, GAP], [1, FW]])
            for _ in range(XBUFS):
                xz = xpool.tile([K, FW], MM_DT)
                nc.sync.dma_start(xz[K0:JSTEP, :], zsrc)
                nc.sync.dma_start(xz[JSTEP + K0: 2 * JSTEP, :], zsrc)

            for n in range(NB):
                for chunk in range(NCHUNK):
                    xin = xpool.tile([K, FW], MM_DT)
                    src = bass.AP(
                        x_d,
                        (n * NCHUNK + chunk) * K0 * FW,
                        [[FW, K0], [1, FW]],
                    )
                    nc.sync.dma_start(xin[:K0, :], src)
                    # j=1,2 replicas: same rows shifted left by j columns.
                    nc.scalar.copy(xin[JSTEP: JSTEP + K0, : FW - 1], xin[:K0, 1:])
                    nc.scalar.copy(xin[2 * JSTEP: 2 * JSTEP + K0, : FW - 2],
                                   xin[:K0, 2:])

                    ob = opool.tile([M, OBF], F32)
                    for half in range(2):
                        ps = ppool.tile([M, T, W_DIM], F32)
                        for t4 in range(T):
                            t = half * T + t4
                            nc.tensor.matmul(
                                ps[:, t4, :],
                                wsb[:],
                                xin[:, Wp * t: Wp * t + W_DIM],
                                start=True,
                                stop=True,
                            )
                        nc.vector.tensor_scalar_add(
                            ob[:, half * T * W_DIM: (half + 1) * T * W_DIM],
                            ps[:, :, :],
                            bsb[:],
                        )

                    dst = bass.AP(
                        out_d,
                        (n * NCHUNK + chunk) * M * OBF,
                        [[OBF, M], [1, OBF]],
                    )
                    nc.sync.dma_start(dst, ob[:])
    nc.finalize()
    return nc


_NC_CACHE = None


def _get_nc():
    global _NC_CACHE
    if _NC_CACHE is None:
        _NC_CACHE = _build_bass()
    return _NC_CACHE


def kernel(x, W, b, trace=False, **trace_kw):
    xin = _build_xin(_round_fp32r(np.asarray(x, np.float32)).reshape(x.shape))
    wt, bias = _build_weights(np.asarray(W, np.float32), np.asarray(b, np.float32))
    wt = _round_fp32r(wt)
    zeros = np.zeros((GAP, FW), np.float32)
    in_maps = [
        {"xin": xin[NB * m: NB * (m + 1)], "wt": wt, "bias": bias, "zeros": zeros}
        for m in range(NCORES)
    ]
    res = run_bass_kernel_spmd(
        _get_nc(), in_maps, list(range(NCORES)), trace=trace, **trace_kw
    )
    # Device layout [NB, NCHUNK, 32g+o, (half,t4,w)] -> [B, OUT, H, W]:
    # row = CH*chunk + 4*(4*half + t4) + g
    parts = []
    for m in range(NCORES):
        o = res.results[m]["out"].reshape(NB, NCHUNK, G, OUT, 2, T, W_DIM)
        parts.append(o.transpose(0, 3, 1, 4, 5, 2, 6).reshape(NB, OUT, H, W_DIM))
    out = np.ascontiguousarray(np.concatenate(parts, axis=0))
    if trace:
        kernel.last_results = res
    return out


# revision 20
# speedup vs baseline: 1.0436x; 1.0436x over previous
"""PointConvolution (8-neighbor shifted diffs + 1x1 conv) as a single 3x3 conv,
run data-parallel across 8 TRN2 NeuronCores via Bass/Tile.

Math: out[o,h,w] = sum_k sum_c W[o,3k+c] * (xpad[c,h+ik,w+jk] - x[c,h,w]) + b[o]
    = sum_{c,i,j} K3[o,c,i,j] * xpad[c,h+i,w+j] + b[o]
  where K3 gets W at the 8 non-center taps and -sum(W over taps) at center.

Device scheme per core (2 images), v4 (K=82, one matmul per row-group):
  - Output rows in chunks of 32 (TB=8 groups of G=4 rows).
  - M=128 PSUM partitions = (g in 0..3, o in 0..31); contraction partitions
    32j + (6c+s) for kernel column j in 0..2, folded in via shifted replicas.
  - Host pre-gathers each chunk's input window into [18, TB*Wp] (im2row over
    rows); one DMA loads it at partitions 0-17, then two ACT copies build the
    j=1,2 replicas at partitions 32/64, shifted 1/2 cols (ACT is the only
    engine that may write float32r, and partition starts must be 32-aligned).
    Gap partitions 18-31/50-63 are zeroed once per pool buffer at startup so
    the matmul (zero weight rows there) never sees NaN garbage.
  - Per group t: ONE self-loading fp32r matmul [82x128]x[82x512] (start+stop).
  - PSUM -> SBUF drain: one DVE tensor_scalar_add per 4-bank half (adds bias),
    then one 128x16KB-descriptor DMA per chunk writes a permuted DRAM layout;
    host transposes during unshard. DMA rings only carry the 9.5MB input +
    67MB output (+ ~214us/core HBM roofline at 358 GB/s).
"""

import numpy as np

import concourse.bacc as bacc
import concourse.bass as bass
import concourse.tile as tile
from concourse import mybir
from concourse.bass_utils import run_bass_kernel_spmd

# Problem constants (hardcoded per harness contract)
B, C, H, W_DIM, OUT = 16, 3, 512, 512, 32
KS, P = 3, 1
NCORES = 8
NB = B // NCORES          # images per core = 2
Hp, Wp = H + 2 * P, W_DIM + 2 * P   # 514, 514

G = 4                     # output rows per matmul group
S = G + KS - 1            # input rows per group window = 6
T = 4                     # groups per PSUM half
TB = 8                    # groups per chunk (32 output rows)
CH = G * TB               # 32 output rows per chunk
NCHUNK = H // CH          # 16 chunks per image
K0 = C * S                # 18 base contraction partitions
JSTEP = 32                # partition stride between j-blocks (alignment)
K = 2 * JSTEP + K0        # 82 contraction partitions incl. zero gaps
GAP = JSTEP - K0          # 14 zero partitions after each of j=0,1 blocks
M = G * OUT               # 128 output partitions
FW = TB * Wp              # 4112 free cols per contraction row
OBF = 2 * T * W_DIM       # 4096 free cols in the output tile
XBUFS = 4                 # xin pool depth (warmup-zeroed once per buffer)

F32 = mybir.dt.float32
MM_DT = mybir.dt.float32r  # PE streams 1 cyc/row (vs 4 for plain fp32)


def _round_fp32r(a):
    """fp32r = fp32 with mantissa rounded (RNE) to 11 bits, low 12 bits zero."""
    bits = np.ascontiguousarray(a, np.float32).view(np.uint32)
    lsb = (bits >> np.uint32(12)) & np.uint32(1)
    out = (bits + np.uint32(0x7FF) + lsb) & np.uint32(0xFFFFF000)
    return out.view(np.float32)


def _coords():
    i, j = np.meshgrid(np.arange(KS), np.arange(KS))
    coords = np.dstack((i.reshape(-1), j.reshape(-1)))[0]
    return coords[np.any(coords != P, axis=1)]


def _build_weights(W, b):
    K3 = np.zeros((OUT, C, KS, KS), np.float32)
    Wr = W.reshape(OUT, 8, C)
    for k, (i, j) in enumerate(_coords()):
        K3[:, :, i, j] += Wr[:, k, :]
    K3[:, :, P, P] = -Wr.sum(axis=1)

    # wt[32j + Sc + s, 32g + o] = K3[o, c, s-g, j] when 0 <= s-g < KS
    wt = np.zeros((K, M), np.float32)
    for j in range(KS):
        for c in range(C):
            for s in range(S):
                for g in range(G):
                    i = s - g
                    if 0 <= i < KS:
                        wt[JSTEP * j + S * c + s, OUT * g: OUT * (g + 1)] = K3[:, c, i, j]
    bias = np.tile(b.astype(np.float32), G).reshape(M, 1)
    return wt, bias


def _build_xin(x):
    """[B,C,H,W] -> [B, NCHUNK, K0, TB*Wp] im2row over rows (padding embedded)."""
    xpad = np.pad(np.ascontiguousarray(x, np.float32),
                  ((0, 0), (0, 0), (P, P), (P, P)))
    ch = np.arange(NCHUNK)[:, None, None]
    s = np.arange(S)[None, :, None]
    t = np.arange(TB)[None, None, :]
    rows = CH * ch + G * t + s                      # [NCHUNK, S, TB]
    big = xpad[:, :, rows, :]                       # [B, C, NCHUNK, S, TB, Wp]
    big = big.transpose(0, 2, 1, 3, 4, 5)           # [B, NCHUNK, C, S, TB, Wp]
    return np.ascontiguousarray(big).reshape(B, NCHUNK, K0, FW)


def _build_bass():
    # Bacc (not plain Bass): its compile() runs move_matmul_waits_to_ldweights
    # and generate_event_semaphores, required because TRN2 instructions take
    # at most one semaphore wait.
    nc = bacc.Bacc("TRN2")
    x_d = nc.declare_dram_parameter("xin", [NB, NCHUNK, K0, FW], MM_DT, isOutput=False)
    wt_d = nc.declare_dram_parameter("wt", [K, M], MM_DT, isOutput=False)
    b_d = nc.declare_dram_parameter("bias", [M, 1], F32, isOutput=False)
    z_d = nc.declare_dram_parameter("zeros", [GAP, FW], MM_DT, isOutput=False)
    out_d = nc.declare_dram_parameter("out", [NB, NCHUNK, M, OBF], F32, isOutput=True)

    with tile.TileContext(nc) as tc:
        with (
            tc.tile_pool(name="wpool", bufs=1) as wpool,
            tc.tile_pool(name="xpool", bufs=XBUFS) as xpool,
            tc.tile_pool(name="opool", bufs=4) as opool,
            tc.tile_pool(name="psum", bufs=2, space=bass.MemorySpace.PSUM) as ppool,
        ):
            wsb = wpool.tile([K, M], MM_DT)
            nc.scalar.dma_start(wsb[:], wt_d[:])
            bsb = wpool.tile([M, 1], F32)
            nc.scalar.dma_start(bsb[:], b_d[:])

            # Zero the gap partitions of each pool buffer once; the loop never
            # writes them, so they stay zero for every later chunk.
            zsrc = bass.AP(z_d, 0, [[FW, GAP], [1, FW]])
            for _ in range(XBUFS):
                xz = xpool.tile([K, FW], MM_DT)
                nc.gpsimd.dma_start(xz[K0:JSTEP, :], zsrc)
                nc.gpsimd.dma_start(xz[JSTEP + K0: 2 * JSTEP, :], zsrc)

            for n in range(NB):
                for chunk in range(NCHUNK):
                    xin = xpool.tile([K, FW], MM_DT)
                    src = bass.AP(
                        x_d,
                        (n * NCHUNK + chunk) * K0 * FW,
                        [[FW, K0], [1, FW]],
                    )
                    nc.gpsimd.dma_start(xin[:K0, :], src)
                    # j=1,2 replicas: same rows shifted left by j columns.
                    nc.scalar.copy(xin[JSTEP: JSTEP + K0, : FW - 1], xin[:K0, 1:])
                    nc.scalar.copy(xin[2 * JSTEP: 2 * JSTEP + K0, : FW - 2],
                                   xin[:K0, 2:])

                    ob = opool.tile([M, OBF], F32)
                    for half in range(2):
                        ps = ppool.tile([M, T, W_DIM], F32)
                        for t4 in range(T):
                            t = half * T + t4
                            nc.tensor.matmul(
                                ps[:, t4, :],
                                wsb[:],
                                xin[:, Wp * t: Wp * t + W_DIM],
                                start=True,
                                stop=True,
                            )
                        nc.vector.tensor_scalar_add(
                            ob[:, half * T * W_DIM: (half + 1) * T * W_DIM],
                            ps[:, :, :],
                            bsb[:],
                        )

                    dst = bass.AP(
                        out_d,
                        (n * NCHUNK + chunk) * M * OBF,
                        [[OBF, M], [1, OBF]],
                    )
                    nc.sync.dma_start(dst, ob[:])
    nc.finalize()
    return nc


_NC_CACHE = None


def _get_nc():
    global _NC_CACHE
    if _NC_CACHE is None:
        _NC_CACHE = _build_bass()
    return _NC_CACHE


def kernel(x, W, b, trace=False, **trace_kw):
    xin = _build_xin(_round_fp32r(np.asarray(x, np.float32)).reshape(x.shape))
    wt, bias = _build_weights(np.asarray(W, np.float32), np.asarray(b, np.float32))
    wt = _round_fp32r(wt)
    zeros = np.zeros((GAP, FW), np.float32)
    in_maps = [
        {"xin": xin[NB * m: NB * (m + 1)], "wt": wt, "bias": bias, "zeros": zeros}
        for m in range(NCORES)
    ]
    res = run_bass_kernel_spmd(
        _get_nc(), in_maps, list(range(NCORES)), trace=trace, **trace_kw
    )
    # Device layout [NB, NCHUNK, 32g+o, (half,t4,w)] -> [B, OUT, H, W]:
    # row = CH*chunk + 4*(4*half + t4) + g
    parts = []
    for m in range(NCORES):
        o = res.results[m]["out"].reshape(NB, NCHUNK, G, OUT, 2, T, W_DIM)
        parts.append(o.transpose(0, 3, 1, 4, 5, 2, 6).reshape(NB, OUT, H, W_DIM))
    out = np.ascontiguousarray(np.concatenate(parts, axis=0))
    if trace:
        kernel.last_results = res
    return out


# revision 22
# speedup vs baseline: 1.2717x; 1.2187x over previous
"""PointConvolution (8-neighbor shifted diffs + 1x1 conv) as a single 3x3 conv,
run data-parallel across 8 TRN2 NeuronCores via Bass/Tile.

Math: out[o,h,w] = sum_k sum_c W[o,3k+c] * (xpad[c,h+ik,w+jk] - x[c,h,w]) + b[o]
    = sum_{c,i,j} K3[o,c,i,j] * xpad[c,h+i,w+j] + b[o]
  where K3 gets W at the 8 non-center taps and -sum(W over taps) at center.

Device scheme per core (2 images), v4 (K=82, one matmul per row-group):
  - Output rows in chunks of 32 (TB=8 groups of G=4 rows).
  - M=128 PSUM partitions = (g in 0..3, o in 0..31); contraction partitions
    32j + (6c+s) for kernel column j in 0..2, folded in via shifted replicas.
  - Host pre-gathers each chunk's input window into [18, TB*Wp] (im2row over
    rows); one DMA loads it at partitions 0-17, then two ACT copies build the
    j=1,2 replicas at partitions 32/64, shifted 1/2 cols (ACT is the only
    engine that may write float32r, and partition starts must be 32-aligned).
    Gap partitions 18-31/50-63 are zeroed once per pool buffer at startup so
    the matmul (zero weight rows there) never sees NaN garbage.
  - Per group t: ONE self-loading fp32r matmul [82x128]x[82x512] (start+stop).
  - PSUM -> SBUF drain: one DVE tensor_scalar_add per 4-bank half (adds bias),
    then one 128x16KB-descriptor DMA per chunk writes a permuted DRAM layout;
    host transposes during unshard. DMA rings only carry the 9.5MB input +
    67MB output (+ ~214us/core HBM roofline at 358 GB/s).
"""

import numpy as np

import concourse.bacc as bacc
import concourse.bass as bass
import concourse.tile as tile
from concourse import mybir
from concourse.bass_utils import run_bass_kernel_spmd

# Problem constants (hardcoded per harness contract)
B, C, H, W_DIM, OUT = 16, 3, 512, 512, 32
KS, P = 3, 1
NCORES = 8
NB = B // NCORES          # images per core = 2
Hp, Wp = H + 2 * P, W_DIM + 2 * P   # 514, 514

G = 4                     # output rows per matmul group
S = G + KS - 1            # input rows per group window = 6
T = 4                     # groups per PSUM half
TB = 8                    # groups per chunk (32 output rows)
CH = G * TB               # 32 output rows per chunk
NCHUNK = H // CH          # 16 chunks per image
K0 = C * S                # 18 base contraction partitions
JSTEP = 32                # partition stride between j-blocks (alignment)
K = 2 * JSTEP + K0        # 82 contraction partitions incl. zero gaps
GAP = JSTEP - K0          # 14 zero partitions after each of j=0,1 blocks
M = G * OUT               # 128 output partitions
FW = TB * Wp              # 4112 free cols per contraction row
OBF = 2 * T * W_DIM       # 4096 free cols in the output tile
XBUFS = 3                 # xin pool depth (warmup-zeroed once per buffer)

F32 = mybir.dt.float32
MM_DT = mybir.dt.float32r  # PE streams 1 cyc/row (vs 4 for plain fp32)


def _round_fp32r(a):
    """fp32r = fp32 with mantissa rounded (RNE) to 11 bits, low 12 bits zero."""
    bits = np.ascontiguousarray(a, np.float32).view(np.uint32)
    lsb = (bits >> np.uint32(12)) & np.uint32(1)
    out = (bits + np.uint32(0x7FF) + lsb) & np.uint32(0xFFFFF000)
    return out.view(np.float32)


def _coords():
    i, j = np.meshgrid(np.arange(KS), np.arange(KS))
    coords = np.dstack((i.reshape(-1), j.reshape(-1)))[0]
    return coords[np.any(coords != P, axis=1)]


def _build_weights(W, b):
    K3 = np.zeros((OUT, C, KS, KS), np.float32)
    Wr = W.reshape(OUT, 8, C)
    for k, (i, j) in enumerate(_coords()):
        K3[:, :, i, j] += Wr[:, k, :]
    K3[:, :, P, P] = -Wr.sum(axis=1)

    # wt[32j + Sc + s, 32g + o] = K3[o, c, s-g, j] when 0 <= s-g < KS
    wt = np.zeros((K, M), np.float32)
    for j in range(KS):
        for c in range(C):
            for s in range(S):
                for g in range(G):
                    i = s - g
                    if 0 <= i < KS:
                        wt[JSTEP * j + S * c + s, OUT * g: OUT * (g + 1)] = K3[:, c, i, j]
    bias = np.tile(b.astype(np.float32), G).reshape(M, 1)
    return wt, bias


def _build_xin(x):
    """[B,C,H,W] -> [B, NCHUNK, K0, TB*Wp] im2row over rows (padding embedded)."""
    xpad = np.pad(np.ascontiguousarray(x, np.float32),
                  ((0, 0), (0, 0), (P, P), (P, P)))
    ch = np.arange(NCHUNK)[:, None, None]
    s = np.arange(S)[None, :, None]
    t = np.arange(TB)[None, None, :]
    rows = CH * ch + G * t + s                      # [NCHUNK, S, TB]
    big = xpad[:, :, rows, :]                       # [B, C, NCHUNK, S, TB, Wp]
    big = big.transpose(0, 2, 1, 3, 4, 5)           # [B, NCHUNK, C, S, TB, Wp]
    return np.ascontiguousarray(big).reshape(B, NCHUNK, K0, FW)


def _build_bass():
    # Bacc (not plain Bass): its compile() runs move_matmul_waits_to_ldweights
    # and generate_event_semaphores, required because TRN2 instructions take
    # at most one semaphore wait.
    nc = bacc.Bacc("TRN2")
    x_d = nc.declare_dram_parameter("xin", [NB, NCHUNK, K0, FW], MM_DT, isOutput=False)
    wt_d = nc.declare_dram_parameter("wt", [K, M], MM_DT, isOutput=False)
    b_d = nc.declare_dram_parameter("bias", [M, 1], F32, isOutput=False)
    z_d = nc.declare_dram_parameter("zeros", [GAP, FW], MM_DT, isOutput=False)
    out_d = nc.declare_dram_parameter("out", [NB, NCHUNK, M, OBF], F32, isOutput=True)

    with tile.TileContext(nc) as tc:
        with (
            tc.tile_pool(name="wpool", bufs=1) as wpool,
            tc.tile_pool(name="xpool", bufs=XBUFS) as xpool,
            tc.tile_pool(name="opool", bufs=3) as opool,
            tc.tile_pool(name="psum", bufs=2, space=bass.MemorySpace.PSUM) as ppool,
        ):
            wsb = wpool.tile([K, M], MM_DT)
            nc.scalar.dma_start(wsb[:], wt_d[:])
            bsb = wpool.tile([M, 1], F32)
            nc.scalar.dma_start(bsb[:], b_d[:])

            # Zero the gap partitions of each pool buffer once; the loop never
            # writes them, so they stay zero for every later chunk.
            zsrc = bass.AP(z_d, 0, [[FW, GAP], [1, FW]])
            for _ in range(XBUFS):
                xz = xpool.tile([K, FW], MM_DT)
                nc.gpsimd.dma_start(xz[K0:JSTEP, :], zsrc)
                nc.gpsimd.dma_start(xz[JSTEP + K0: 2 * JSTEP, :], zsrc)

            for n in range(NB):
                for chunk in range(NCHUNK):
                    xin = xpool.tile([K, FW], MM_DT)
                    src = bass.AP(
                        x_d,
                        (n * NCHUNK + chunk) * K0 * FW,
                        [[FW, K0], [1, FW]],
                    )
                    nc.gpsimd.dma_start(xin[:K0, :], src)
                    # j=1,2 replicas: same rows shifted left by j columns.
                    nc.scalar.copy(xin[JSTEP: JSTEP + K0, : FW - 1], xin[:K0, 1:])
                    nc.scalar.copy(xin[2 * JSTEP: 2 * JSTEP + K0, : FW - 2],
                                   xin[:K0, 2:])

                    ob = opool.tile([M, OBF], F32)
                    for half in range(2):
                        ps = ppool.tile([M, T, W_DIM], F32)
                        for t4 in range(T):
                            t = half * T + t4
                            nc.tensor.matmul(
                                ps[:, t4, :],
                                wsb[:],
                                xin[:, Wp * t: Wp * t + W_DIM],
                                start=True,
                                stop=True,
                            )
                        nc.vector.tensor_scalar_add(
                            ob[:, half * T * W_DIM: (half + 1) * T * W_DIM],
                            ps[:, :, :],
                            bsb[:],
                        )

                    dst = bass.AP(
                        out_d,
                        (n * NCHUNK + chunk) * M * OBF,
                        [[OBF, M], [1, OBF]],
                    )
                    nc.sync.dma_start(dst, ob[:])
    nc.finalize()
    return nc


_NC_CACHE = None


def _get_nc():
    global _NC_CACHE
    if _NC_CACHE is None:
        _NC_CACHE = _build_bass()
    return _NC_CACHE


def kernel(x, W, b, trace=False, **trace_kw):
    xin = _build_xin(_round_fp32r(np.asarray(x, np.float32)).reshape(x.shape))
    wt, bias = _build_weights(np.asarray(W, np.float32), np.asarray(b, np.float32))
    wt = _round_fp32r(wt)
    zeros = np.zeros((GAP, FW), np.float32)
    in_maps = [
        {"xin": xin[NB * m: NB * (m + 1)], "wt": wt, "bias": bias, "zeros": zeros}
        for m in range(NCORES)
    ]
    res = run_bass_kernel_spmd(
        _get_nc(), in_maps, list(range(NCORES)), trace=trace, **trace_kw
    )
    # Device layout [NB, NCHUNK, 32g+o, (half,t4,w)] -> [B, OUT, H, W]:
    # row = CH*chunk + 4*(4*half + t4) + g
    parts = []
    for m in range(NCORES):
        o = res.results[m]["out"].reshape(NB, NCHUNK, G, OUT, 2, T, W_DIM)
        parts.append(o.transpose(0, 3, 1, 4, 5, 2, 6).reshape(NB, OUT, H, W_DIM))
    out = np.ascontiguousarray(np.concatenate(parts, axis=0))
    if trace:
        kernel.last_results = res
    return out


# revision 24
# speedup vs baseline: 1.4629x; 1.1503x over previous
"""PointConvolution (8-neighbor shifted diffs + 1x1 conv) as a single 3x3 conv,
run data-parallel across 8 TRN2 NeuronCores via Bass/Tile.

Math: out[o,h,w] = sum_k sum_c W[o,3k+c] * (xpad[c,h+ik,w+jk] - x[c,h,w]) + b[o]
    = sum_{c,i,j} K3[o,c,i,j] * xpad[c,h+i,w+j] + b[o]
  where K3 gets W at the 8 non-center taps and -sum(W over taps) at center.

Device scheme per core (2 images), v4 (K=82, one matmul per row-group):
  - Output rows in chunks of 32 (TB=8 groups of G=4 rows).
  - M=128 PSUM partitions = (g in 0..3, o in 0..31); contraction partitions
    32j + (6c+s) for kernel column j in 0..2, folded in via shifted replicas.
  - Host pre-gathers each chunk's input window into [18, TB*Wp] (im2row over
    rows); one DMA loads it at partitions 0-17, then two ACT copies build the
    j=1,2 replicas at partitions 32/64, shifted 1/2 cols (ACT is the only
    engine that may write float32r, and partition starts must be 32-aligned).
    Gap partitions 18-31/50-63 are zeroed once per pool buffer at startup so
    the matmul (zero weight rows there) never sees NaN garbage.
  - Per group t: ONE self-loading fp32r matmul [82x128]x[82x512] (start+stop).
  - PSUM -> SBUF drain: one DVE tensor_scalar_add per 4-bank half (adds bias),
    then one 128x16KB-descriptor DMA per chunk writes a permuted DRAM layout;
    host transposes during unshard. DMA rings only carry the 9.5MB input +
    67MB output (+ ~214us/core HBM roofline at 358 GB/s).
"""

import numpy as np

import concourse.bacc as bacc
import concourse.bass as bass
import concourse.tile as tile
from concourse import mybir
from concourse.bass_utils import run_bass_kernel_spmd

# Problem constants (hardcoded per harness contract)
B, C, H, W_DIM, OUT = 16, 3, 512, 512, 32
KS, P = 3, 1
NCORES = 8
NB = B // NCORES          # images per core = 2
Hp, Wp = H + 2 * P, W_DIM + 2 * P   # 514, 514

G = 4                     # output rows per matmul group
S = G + KS - 1            # input rows per group window = 6
T = 4                     # groups per PSUM half
TB = 8                    # groups per chunk (32 output rows)
CH = G * TB               # 32 output rows per chunk
NCHUNK = H // CH          # 16 chunks per image
K0 = C * S                # 18 base contraction partitions
JSTEP = 32                # partition stride between j-blocks (alignment)
K = 2 * JSTEP + K0        # 82 contraction partitions incl. zero gaps
GAP = JSTEP - K0          # 14 zero partitions after each of j=0,1 blocks
M = G * OUT               # 128 output partitions
FW = TB * Wp              # 4112 free cols per contraction row
OBF = 2 * T * W_DIM       # 4096 free cols in the output tile
XBUFS = 3                 # xin pool depth (warmup-zeroed once per buffer)
SPLIT = 2720              # cols of the j=2 replica copied on ACT (rest on DVE)

F32 = mybir.dt.float32
MM_DT = mybir.dt.float32r  # PE streams 1 cyc/row (vs 4 for plain fp32)


def _round_fp32r(a):
    """fp32r = fp32 with mantissa rounded (RNE) to 11 bits, low 12 bits zero."""
    bits = np.ascontiguousarray(a, np.float32).view(np.uint32)
    lsb = (bits >> np.uint32(12)) & np.uint32(1)
    out = (bits + np.uint32(0x7FF) + lsb) & np.uint32(0xFFFFF000)
    return out.view(np.float32)


def _coords():
    i, j = np.meshgrid(np.arange(KS), np.arange(KS))
    coords = np.dstack((i.reshape(-1), j.reshape(-1)))[0]
    return coords[np.any(coords != P, axis=1)]


def _build_weights(W, b):
    K3 = np.zeros((OUT, C, KS, KS), np.float32)
    Wr = W.reshape(OUT, 8, C)
    for k, (i, j) in enumerate(_coords()):
        K3[:, :, i, j] += Wr[:, k, :]
    K3[:, :, P, P] = -Wr.sum(axis=1)

    # wt[32j + Sc + s, 32g + o] = K3[o, c, s-g, j] when 0 <= s-g < KS
    wt = np.zeros((K, M), np.float32)
    for j in range(KS):
        for c in range(C):
            for s in range(S):
                for g in range(G):
                    i = s - g
                    if 0 <= i < KS:
                        wt[JSTEP * j + S * c + s, OUT * g: OUT * (g + 1)] = K3[:, c, i, j]
    bias = np.tile(b.astype(np.float32), G).reshape(M, 1)
    return wt, bias


def _build_xin(x):
    """[B,C,H,W] -> [B, NCHUNK, K0, TB*Wp] im2row over rows (padding embedded)."""
    xpad = np.pad(np.ascontiguousarray(x, np.float32),
                  ((0, 0), (0, 0), (P, P), (P, P)))
    ch = np.arange(NCHUNK)[:, None, None]
    s = np.arange(S)[None, :, None]
    t = np.arange(TB)[None, None, :]
    rows = CH * ch + G * t + s                      # [NCHUNK, S, TB]
    big = xpad[:, :, rows, :]                       # [B, C, NCHUNK, S, TB, Wp]
    big = big.transpose(0, 2, 1, 3, 4, 5)           # [B, NCHUNK, C, S, TB, Wp]
    return np.ascontiguousarray(big).reshape(B, NCHUNK, K0, FW)


def _build_bass():
    # Bacc (not plain Bass): its compile() runs move_matmul_waits_to_ldweights
    # and generate_event_semaphores, required because TRN2 instructions take
    # at most one semaphore wait.
    nc = bacc.Bacc("TRN2")
    x_d = nc.declare_dram_parameter("xin", [NB, NCHUNK, K0, FW], MM_DT, isOutput=False)
    wt_d = nc.declare_dram_parameter("wt", [K, M], MM_DT, isOutput=False)
    b_d = nc.declare_dram_parameter("bias", [M, 1], F32, isOutput=False)
    z_d = nc.declare_dram_parameter("zeros", [GAP, FW], MM_DT, isOutput=False)
    out_d = nc.declare_dram_parameter("out", [NB, NCHUNK, M, OBF], F32, isOutput=True)

    with tile.TileContext(nc) as tc:
        with (
            tc.tile_pool(name="wpool", bufs=1) as wpool,
            tc.tile_pool(name="xpool", bufs=XBUFS) as xpool,
            tc.tile_pool(name="opool", bufs=3) as opool,
            tc.tile_pool(name="psum", bufs=2, space=bass.MemorySpace.PSUM) as ppool,
        ):
            wsb = wpool.tile([K, M], MM_DT)
            nc.scalar.dma_start(wsb[:], wt_d[:])
            bsb = wpool.tile([M, 1], F32)
            nc.scalar.dma_start(bsb[:], b_d[:])

            # Zero the gap partitions of each pool buffer once; the loop never
            # writes them, so they stay zero for every later chunk.
            zsrc = bass.AP(z_d, 0, [[FW, GAP], [1, FW]])
            for _ in range(XBUFS):
                xz = xpool.tile([K, FW], MM_DT)
                nc.gpsimd.dma_start(xz[K0:JSTEP, :], zsrc)
                nc.gpsimd.dma_start(xz[JSTEP + K0: 2 * JSTEP, :], zsrc)

            for n in range(NB):
                for chunk in range(NCHUNK):
                    xin = xpool.tile([K, FW], MM_DT)
                    src = bass.AP(
                        x_d,
                        (n * NCHUNK + chunk) * K0 * FW,
                        [[FW, K0], [1, FW]],
                    )
                    nc.gpsimd.dma_start(xin[:K0, :], src)
                    # j=1,2 replicas: same rows shifted left by j columns.
                    # j=2 is split ACT/DVE to balance engine load (DVE "copy"
                    # = tensor_scalar_add with immediate 0).
                    nc.scalar.copy(xin[JSTEP: JSTEP + K0, : FW - 1], xin[:K0, 1:])
                    nc.scalar.copy(xin[2 * JSTEP: 2 * JSTEP + K0, :SPLIT],
                                   xin[:K0, 2: 2 + SPLIT])
                    nc.vector.tensor_scalar_add(
                        xin[2 * JSTEP: 2 * JSTEP + K0, SPLIT: FW - 2],
                        xin[:K0, SPLIT + 2: FW],
                        0.0,
                    )

                    ob = opool.tile([M, OBF], F32)
                    for half in range(2):
                        ps = ppool.tile([M, T, W_DIM], F32)
                        for t4 in range(T):
                            t = half * T + t4
                            nc.tensor.matmul(
                                ps[:, t4, :],
                                wsb[:],
                                xin[:, Wp * t: Wp * t + W_DIM],
                                start=True,
                                stop=True,
                            )
                        nc.vector.tensor_scalar_add(
                            ob[:, half * T * W_DIM: (half + 1) * T * W_DIM],
                            ps[:, :, :],
                            bsb[:],
                        )

                    dst = bass.AP(
                        out_d,
                        (n * NCHUNK + chunk) * M * OBF,
                        [[OBF, M], [1, OBF]],
                    )
                    nc.sync.dma_start(dst, ob[:])
    nc.finalize()
    return nc


_NC_CACHE = None


def _get_nc():
    global _NC_CACHE
    if _NC_CACHE is None:
        _NC_CACHE = _build_bass()
    return _NC_CACHE


def kernel(x, W, b, trace=False, **trace_kw):
    xin = _build_xin(_round_fp32r(np.asarray(x, np.float32)).reshape(x.shape))
    wt, bias = _build_weights(np.asarray(W, np.float32), np.asarray(b, np.float32))
    wt = _round_fp32r(wt)
    zeros = np.zeros((GAP, FW), np.float32)
    in_maps = [
        {"xin": xin[NB * m: NB * (m + 1)], "wt": wt, "bias": bias, "zeros": zeros}
        for m in range(NCORES)
    ]
    res = run_bass_kernel_spmd(
        _get_nc(), in_maps, list(range(NCORES)), trace=trace, **trace_kw
    )
    # Device layout [NB, NCHUNK, 32g+o, (half,t4,w)] -> [B, OUT, H, W]:
    # row = CH*chunk + 4*(4*half + t4) + g
    parts = []
    for m in range(NCORES):
        o = res.results[m]["out"].reshape(NB, NCHUNK, G, OUT, 2, T, W_DIM)
        parts.append(o.transpose(0, 3, 1, 4, 5, 2, 6).reshape(NB, OUT, H, W_DIM))
    out = np.ascontiguousarray(np.concatenate(parts, axis=0))
    if trace:
        kernel.last_results = res
    return out
